# revision 1
# baseline (speedup 1.0000x reference)
"""GQA causal attention (B=2, T=2048, D=2048, N=16 q-heads, K=4 kv-heads, H=128)
on 8 Trainium2 NeuronCores.

Sharding: core c -> (batch b = c // 4, kv-head g = c % 4). Each core owns one
batch element and one GQA group (1 kv head + its 4 query heads) and computes
the full pipeline for that shard: Q/K/V projections, RoPE, causal SDPA, and
the O-projection partial over its 4 heads. The host pre-transposes activations
to [D, T], precomputes RoPE sin/cos tables and causal mask tiles, and sums the
4 per-core O-projection partials of each batch afterwards.

Device design notes:
  - qT/kT live [head_dim, T] (head dim on partitions); V is [s, H] via PE
    transpose; scores are computed directly transposed [s, t], so the softmax
    needs no on-chip transposes: exp without max-subtraction (logits are O(5)
    here), row sums via ones-vector matmuls, normalization after PV.
  - Matmuls in float32r: full PE rate at free-dim 512, ~1.5e-4 rel err.
  - DMAs are consolidated with 3D access patterns (one descriptor-generation
    pass per tensor-chunk instead of per 128-partition tile): HWDGE setup is
    ~0.6us per dma_start, so DMA count dominates the transfer schedule.
"""

import sys

for _p in ("/opt/trn_rl_repo", "/root/.axon_site/_ro/trn_rl_repo"):
    if _p not in sys.path:
        sys.path.append(_p)

import numpy as np

import concourse.bass as bass
import concourse.mybir as mybir
import concourse.tile as tile
from concourse import bacc
from concourse.bass_utils import run_bass_kernel_spmd
from concourse.masks import make_identity

B, T, D = 2, 2048, 2048
N_HEADS, K_HEADS, H = 16, 4, 128
GH = N_HEADS // K_HEADS          # 4 query heads per core
MIN_TS, MAX_TS = 1.0, 10000.0
NJ = T // 512                    # 4 column chunks of 512
ND = D // 128                    # 16 contraction chunks
SCALE = 1.0 / float(np.sqrt(H))
NEG = -3.0e37

F32 = mybir.dt.float32
MM_FP32R = True
MMDT = mybir.dt.float32r if MM_FP32R else F32

XG = 4    # d-chunks per X dma_start (1, 2, 4, 8, or 16)
OG = 2    # 512-wide output blocks per store dma_start (1 or 2)

_CACHED_NC = None
_last_in_maps = None


def _build_core_program():
    nc = bacc.Bacc("TRN2", target_bir_lowering=False, debug=False, num_devices=8)

    xqT = nc.dram_tensor("xqT", [D, T], MMDT, kind="ExternalInput").ap()
    xkvT = nc.dram_tensor("xkvT", [D, T], MMDT, kind="ExternalInput").ap()
    wq = nc.dram_tensor("wq", [D, GH * H], MMDT, kind="ExternalInput").ap()
    wk = nc.dram_tensor("wk", [D, H], MMDT, kind="ExternalInput").ap()
    wv = nc.dram_tensor("wv", [D, H], MMDT, kind="ExternalInput").ap()
    wo = nc.dram_tensor("wo", [GH * H, D], MMDT, kind="ExternalInput").ap()
    tabs = nc.dram_tensor("tabs", [128, 4 * T], F32, kind="ExternalInput").ap()
    out = nc.dram_tensor("out", [T, D], F32, kind="ExternalOutput").ap()

    with tile.TileContext(nc) as tc:
        _emit(tc, nc, xqT, xkvT, wq, wk, wv, wo, tabs, out)
    nc.compile()
    return nc


def _emit(tc, nc, xqT, xkvT, wq, wk, wv, wo, tabs, out):
    from contextlib import ExitStack

    # 3D source views: [partition 128, d-chunk, col]
    xq_src = xqT.rearrange("(kd p) t -> p kd t", p=128)
    xkv_src = xkvT.rearrange("(kd p) t -> p kd t", p=128)
    wq_src = wq.rearrange("(kd p) n -> p kd n", p=128)
    wk_src = wk.rearrange("(kd p) n -> p kd n", p=128)
    wv_src = wv.rearrange("(kd p) n -> p kd n", p=128)
    wo_src = wo.rearrange("(h p) d -> p h d", p=128)
    tab_src = tabs.rearrange("p (i t) -> p i t", i=4)

    with ExitStack() as ctx:
        const = ctx.enter_context(tc.tile_pool(name="const", bufs=1))
        xq_pool = ctx.enter_context(tc.tile_pool(name="xq", bufs=1))
        xkv_pool = ctx.enter_context(tc.tile_pool(name="xkv", bufs=2))
        tab_pool = ctx.enter_context(tc.tile_pool(name="tab", bufs=1))
        qrot_pool = ctx.enter_context(tc.tile_pool(name="qrot", bufs=4))
        attnt_pool = ctx.enter_context(tc.tile_pool(name="attnt", bufs=4))
        probs_pool = ctx.enter_context(tc.tile_pool(name="probs", bufs=3))
        work = ctx.enter_context(tc.tile_pool(name="work", bufs=2))
        osb_pool = ctx.enter_context(tc.tile_pool(name="osb", bufs=2))
        psum = ctx.enter_context(tc.tile_pool(name="psum", bufs=5, space="PSUM"))
        psum_attn = ctx.enter_context(
            tc.tile_pool(name="psum_attn", bufs=2, space="PSUM")
        )
        psum_lrow = ctx.enter_context(
            tc.tile_pool(name="psum_lrow", bufs=1, space="PSUM")
        )

        ident = const.tile([128, 128], F32, tag="ident")
        make_identity(nc, ident[:])
        ones_f = work.tile([128, 1], F32, tag="scratch", name="ones_f")
        nc.vector.memset(ones_f[:], 1.0)
        ones_s = const.tile([128, 1], MMDT, tag="ones_s")
        nc.vector.tensor_copy(ones_s[:], ones_f[:])
        ones_1f = work.tile([1, 128], F32, tag="scratch", name="ones_1f")
        nc.vector.memset(ones_1f[:], 1.0)
        ones_1 = const.tile([1, 128], MMDT, tag="ones_1")
        nc.vector.tensor_copy(ones_1[:], ones_1f[:])

        krot_sb = const.tile([128, T], MMDT, tag="krot")
        v_sb = const.tile([128, T], MMDT, tag="v")
        wq_all = const.tile([128, ND, GH * H], MMDT, tag="wq")
        wk_all = const.tile([128, ND, H], MMDT, tag="wk")
        wv_all = const.tile([128, ND, H], MMDT, tag="wv")
        wo_all = const.tile([128, GH, D], MMDT, tag="wo")
        # fp8e5 mask: -49152 is representable and exp(SCALE*(x-49152)) == 0
        mask_all = const.tile([128, 4, 512], mybir.dt.float8e5, tag="mask")
        for o in range(4):
            nc.gpsimd.memset(mask_all[:, o, :], 0.0)
            # additive causal mask: 0 where t_local - s_local - o*128 >= 0
            nc.gpsimd.affine_select(
                out=mask_all[:, o, :], in_=mask_all[:, o, :],
                compare_op=mybir.AluOpType.is_ge,
                fill=-49152.0, base=-o * 128,
                pattern=[[1, 512]], channel_multiplier=-1)

        def rope(dst, src_psum, cc_t, ss_t):
            # dst = src * cc + swap_halves(src) * ss
            tmp1 = work.tile([128, 512], F32, tag="scratch")
            tmp2 = work.tile([128, 512], F32, tag="scratch")
            nc.vector.tensor_mul(tmp1[0:64, :], src_psum[64:128, :], ss_t[0:64, :])
            nc.vector.tensor_mul(tmp1[64:128, :], src_psum[0:64, :], ss_t[64:128, :])
            nc.vector.tensor_mul(tmp2[:], src_psum[:], cc_t[:])
            nc.vector.tensor_add(dst, tmp1[:], tmp2[:])

        for J in range(NJ):
            tsl = slice(J * 512, (J + 1) * 512)

            xq_t = xq_pool.tile([128, ND, 512], MMDT, tag="xq", name=f"xq{J}")
            if J == 0:
                # split the J0 prologue so the first Q matmuls' inputs land
                # first; interleave wq with xq per quarter
                for q in range(4):
                    qs = slice(q * 4, (q + 1) * 4)
                    nc.sync.dma_start(xq_t[:, qs, :], xq_src[:, qs, tsl])
                    nc.sync.dma_start(wq_all[:, qs, :], wq_src[:, qs, :])
            else:
                for q in range(ND // XG):
                    qs = slice(q * XG, (q + 1) * XG)
                    nc.sync.dma_start(xq_t[:, qs, :], xq_src[:, qs, tsl])
            xkv_halves = []
            if J == 0:
                nc.sync.dma_start(wk_all[:], wk_src[:])
                nc.sync.dma_start(wv_all[:], wv_src[:])
            for q in range(2):
                xh = xkv_pool.tile([128, 8, 512], MMDT, tag="xkv",
                                   name=f"xkv{J}_{q}")
                nc.sync.dma_start(xh[:], xkv_src[:, q * 8:(q + 1) * 8, tsl])
                xkv_halves.append(xh)
            tab_t = tab_pool.tile([128, 4, 512], F32, tag="tab", name=f"tab{J}")
            nc.sync.dma_start(tab_t[:], tab_src[:, :, tsl])
            ccq_t, ssq_t = tab_t[:, 0, :], tab_t[:, 1, :]
            cck_t, ssk_t = tab_t[:, 2, :], tab_t[:, 3, :]

            # ---- Q projection: sequential heads ----
            qrot = []
            for h in range(GH):
                qps = psum.tile([128, 512], F32, tag="mm", name=f"qps{J}_{h}")
                for kd in range(ND):
                    nc.tensor.matmul(
                        qps[:], wq_all[:, kd, h * 128:(h + 1) * 128],
                        xq_t[:, kd, :], start=(kd == 0), stop=(kd == ND - 1),
                        skip_group_check=True)
                qr = qrot_pool.tile([128, 512], MMDT, tag="qrot",
                                    name=f"qrot{J}_{h}")
                rope(qr[:], qps[:], ccq_t, ssq_t)
                qrot.append(qr)

            # ---- K and V projections ----
            kps = psum.tile([128, 512], F32, tag="mm")
            vps = psum.tile([128, 512], F32, tag="mm")
            for q in range(2):
                xkv_t = xkv_halves[q]
                for kq in range(8):
                    kd = q * 8 + kq
                    nc.tensor.matmul(kps[:], wk_all[:, kd, :], xkv_t[:, kq, :],
                                     start=(kd == 0), stop=(kd == ND - 1),
                                     skip_group_check=True)
                    nc.tensor.matmul(vps[:], wv_all[:, kd, :], xkv_t[:, kq, :],
                                     start=(kd == 0), stop=(kd == ND - 1),
                                     skip_group_check=True)
            rope(krot_sb[:, tsl], kps[:], cck_t, ssk_t)

            # V: [h, t] -> PE-transpose -> v_sb [s, h]
            vt_sb = work.tile([128, 512], F32, tag="scratch")
            nc.vector.tensor_copy(vt_sb[:], vps[:])
            for st in range(4):
                tp = psum.tile([128, 128], F32, tag="mm")
                nc.tensor.transpose(tp[:], vt_sb[:, st * 128:(st + 1) * 128],
                                    ident[:])
                s_tile = J * 4 + st
                nc.vector.tensor_copy(
                    v_sb[:, s_tile * 128:(s_tile + 1) * 128], tp[:])

            if J == 0:
                nc.sync.dma_start(wo_all[:], wo_src[:])

            # ---- SDPA for chunk J, all 4 heads ----
            attnT = []
            nk = 4 * J + 4
            for h in range(GH):
                attn_ps = psum_attn.tile([128, 512], F32, tag="attn")
                lrow_ps = psum_lrow.tile([1, 512], F32, tag="lrow")
                for k0 in range(0, nk, 3):
                    kb = range(k0, min(k0 + 3, nk))
                    pts = []
                    for k in kb:
                        sc = psum.tile([128, 512], F32, tag="mm",
                                       name=f"sc{J}_{h}_{k}")
                        nc.tensor.matmul(sc[:], krot_sb[:, k * 128:(k + 1) * 128],
                                         qrot[h][:], start=True, stop=True)
                        if k >= 4 * J:
                            nc.vector.tensor_add(sc[:], sc[:],
                                                 mask_all[:, k - 4 * J, :])
                        pt = probs_pool.tile([128, 512], MMDT, tag="probs",
                                             name=f"pt{J}_{h}_{k}")
                        nc.scalar.activation(pt[:], sc[:],
                                             mybir.ActivationFunctionType.Exp,
                                             scale=SCALE)
                        pts.append((k, pt))
                    for k, pt in pts:
                        nc.tensor.matmul(attn_ps[:],
                                         v_sb[:, k * 128:(k + 1) * 128], pt[:],
                                         start=(k == 0), stop=(k == nk - 1))
                        nc.tensor.matmul(lrow_ps[:], ones_s[:], pt[:],
                                         start=(k == 0), stop=(k == nk - 1))
                lrow_sb = work.tile([1, 512], MMDT, tag="lrow", bufs=1)
                nc.vector.tensor_copy(lrow_sb[:], lrow_ps[:])
                lbc = psum.tile([128, 512], F32, tag="mm")
                nc.tensor.matmul(lbc[:], ones_1[:], lrow_sb[:],
                                 start=True, stop=True)
                lbc_sb = work.tile([128, 512], F32, tag="scratch")
                nc.vector.reciprocal_approx_fast(lbc_sb[:], lbc[:])
                at = attnt_pool.tile([128, 512], MMDT, tag="attnt")
                nc.vector.tensor_mul(at[:], attn_ps[:], lbc_sb[:])
                attnT.append(at)

            # ---- O projection for chunk J ----
            for tt in range(4):
                csl = slice(tt * 128, (tt + 1) * 128)
                for djp in range(4 // OG):  # OG d-chunks -> one DMA each
                    ot = osb_pool.tile([128, OG * 512], F32, tag="osb",
                                       name=f"ot{J}_{tt}_{djp}")
                    for dje in range(OG):
                        dj = djp * OG + dje
                        ops = psum.tile([128, 512], F32, tag="mm")
                        for h in range(GH):
                            nc.tensor.matmul(
                                ops[:], attnT[h][:, csl],
                                wo_all[:, h, dj * 512:(dj + 1) * 512],
                                start=(h == 0), stop=(h == GH - 1))
                        nc.scalar.copy(
                            ot[:, dje * 512:(dje + 1) * 512], ops[:])
                    # ACT-triggered HWDGE: output stores stay out of the
                    # sync-engine input-prefetch stream
                    nc.scalar.dma_start(
                        out[J * 512 + tt * 128:J * 512 + (tt + 1) * 128,
                            djp * OG * 512:(djp + 1) * OG * 512], ot[:])


def _rope_tables(positions):
    # positions: [T] int -> cc [128, T] = [cos; cos], ss [128, T] = [-sin; sin]
    half = H // 2
    fraction = 2.0 * np.arange(half, dtype=np.float64) / H
    timescale = MIN_TS * (MAX_TS / MIN_TS) ** fraction
    sinusoid = positions.astype(np.float64)[None, :] / timescale[:, None]
    sin = np.sin(sinusoid)
    cos = np.cos(sinusoid)
    cc = np.concatenate([cos, cos], axis=0).astype(np.float32)
    ss = np.concatenate([-sin, sin], axis=0).astype(np.float32)
    return cc, ss


def kernel(Xq, Xkv, q_positions, kv_positions, Wq, Wk, Wv, Wo):
    global _CACHED_NC, _last_in_maps
    if _CACHED_NC is None:
        _CACHED_NC = _build_core_program()
    nc = _CACHED_NC

    Xq = np.asarray(Xq, dtype=np.float32)
    Xkv = np.asarray(Xkv, dtype=np.float32)
    Wq = np.asarray(Wq, dtype=np.float32)
    Wk = np.asarray(Wk, dtype=np.float32)
    Wv = np.asarray(Wv, dtype=np.float32)
    Wo = np.asarray(Wo, dtype=np.float32)
    q_positions = np.asarray(q_positions)
    kv_positions = np.asarray(kv_positions)

    in_maps = []
    for c in range(8):
        b, g = c // 4, c % 4
        ccq, ssq = _rope_tables(q_positions[b])
        cck, ssk = _rope_tables(kv_positions[b])
        tabs = np.ascontiguousarray(
            np.concatenate([ccq, ssq, cck, ssk], axis=1))
        in_maps.append({
            "xqT": np.ascontiguousarray(Xq[b].T),
            "xkvT": np.ascontiguousarray(Xkv[b].T),
            "wq": np.ascontiguousarray(
                Wq[:, g * GH:(g + 1) * GH, :].reshape(D, GH * H)),
            "wk": np.ascontiguousarray(Wk[:, g, :]),
            "wv": np.ascontiguousarray(Wv[:, g, :]),
            "wo": np.ascontiguousarray(Wo[g * GH:(g + 1) * GH].reshape(GH * H, D)),
            "tabs": tabs,
        })

    _last_in_maps = in_maps

    res = run_bass_kernel_spmd(nc, in_maps, list(range(8)))

    outp = np.zeros((B, T, D), dtype=np.float64)
    for c in range(8):
        outp[c // 4] += res.results[c]["out"].astype(np.float64)
    return outp.astype(np.float32)



# revision 3
# speedup vs baseline: 3.4961x; 3.4961x over previous
"""GQA causal attention (B=2, T=2048, D=2048, N=16 q-heads, K=4 kv-heads, H=128)
on 8 Trainium2 NeuronCores.

Sharding: core c -> (batch b = c // 4, kv-head g = c % 4). Each core owns one
batch element and one GQA group (1 kv head + its 4 query heads) and computes
the full pipeline for that shard: Q/K/V projections, RoPE, causal SDPA, and
the O-projection partial over its 4 heads. The host pre-transposes activations
to [D, T] in bf16, precomputes RoPE sin/cos tables, and sums the 4 per-core
O-projection partials of each batch afterwards.

Device design notes (all matmuls bf16, 1 cycle/row on the PE at any free dim):
  - qT/kT live [head_dim, T]; scores are computed transposed [s, t] so the
    softmax denominator and PV contraction both run over the partition dim.
  - Causal masking: diagonal score tiles compute only the live columns
    (free-dim offset) and the probs triangle is zeroed by a Pool-engine
    affine_select after the exp - no mask adds on the Vector engine.
  - PV is computed with probs as the stationary operand and V~ = [V^T | 1]
    (129 columns) as the moving operand, so each [t,129] PSUM tile carries
    the attention numerator AND the softmax denominator in column 128:
    no separate row-sum matmuls. Normalization is a per-partition
    reciprocal+scale on the [t,h] tile, then a PE transpose back to [h,t]
    for the O projection.
  - DMAs are consolidated with 3D access patterns; output stores go through
    ACT-triggered HWDGE to stay out of the sync-engine input stream.
"""

import sys

for _p in ("/opt/trn_rl_repo", "/root/.axon_site/_ro/trn_rl_repo"):
    if _p not in sys.path:
        sys.path.append(_p)

import numpy as np

import concourse.bass as bass
import concourse.mybir as mybir
import concourse.tile as tile
from concourse import bacc
from concourse.bass_utils import run_bass_kernel_spmd
from concourse.masks import make_identity

B, T, D = 2, 2048, 2048
N_HEADS, K_HEADS, H = 16, 4, 128
GH = N_HEADS // K_HEADS          # 4 query heads per core
MIN_TS, MAX_TS = 1.0, 10000.0
NJ = T // 512                    # 4 column chunks of 512
ND = D // 128                    # 16 contraction chunks
SCALE = 1.0 / float(np.sqrt(H))

F32 = mybir.dt.float32
BF16 = mybir.dt.bfloat16
MMDT = BF16

XG = 4    # d-chunks per X dma_start
VW = 132  # v-tile stride: 128 V^T cols + 1 ones col + 3 pad

_CACHED_NC = None
_last_in_maps = None


def _build_core_program():
    nc = bacc.Bacc("TRN2", target_bir_lowering=False, debug=False, num_devices=8)

    xqT = nc.dram_tensor("xqT", [D, T], MMDT, kind="ExternalInput").ap()
    xkvT = nc.dram_tensor("xkvT", [D, T], MMDT, kind="ExternalInput").ap()
    wq = nc.dram_tensor("wq", [D, GH * H], MMDT, kind="ExternalInput").ap()
    wk = nc.dram_tensor("wk", [D, H], MMDT, kind="ExternalInput").ap()
    wv = nc.dram_tensor("wv", [D, H], MMDT, kind="ExternalInput").ap()
    wo = nc.dram_tensor("wo", [GH * H, D], MMDT, kind="ExternalInput").ap()
    tabs = nc.dram_tensor("tabs", [128, 4 * T], F32, kind="ExternalInput").ap()
    out = nc.dram_tensor("out", [T, D], F32, kind="ExternalOutput").ap()

    with tile.TileContext(nc) as tc:
        _emit(tc, nc, xqT, xkvT, wq, wk, wv, wo, tabs, out)
    nc.compile()
    return nc


def _emit(tc, nc, xqT, xkvT, wq, wk, wv, wo, tabs, out):
    from contextlib import ExitStack

    # 3D source views: [partition 128, d-chunk, col]
    xq_src = xqT.rearrange("(kd p) t -> p kd t", p=128)
    xkv_src = xkvT.rearrange("(kd p) t -> p kd t", p=128)
    wq_src = wq.rearrange("(kd p) n -> p kd n", p=128)
    wk_src = wk.rearrange("(kd p) n -> p kd n", p=128)
    wv_src = wv.rearrange("(kd p) n -> p kd n", p=128)
    wo_src = wo.rearrange("(h p) d -> p h d", p=128)
    tab_src = tabs.rearrange("p (i t) -> p i t", i=4)

    with ExitStack() as ctx:
        const = ctx.enter_context(tc.tile_pool(name="const", bufs=1))
        xq_pool = ctx.enter_context(tc.tile_pool(name="xq", bufs=1))
        xkv_pool = ctx.enter_context(tc.tile_pool(name="xkv", bufs=2))
        tab_pool = ctx.enter_context(tc.tile_pool(name="tab", bufs=1))
        qrot_pool = ctx.enter_context(tc.tile_pool(name="qrot", bufs=4))
        attnt_pool = ctx.enter_context(tc.tile_pool(name="attnt", bufs=5))
        probs_pool = ctx.enter_context(tc.tile_pool(name="probs", bufs=18))
        work = ctx.enter_context(tc.tile_pool(name="work", bufs=4))
        norm_pool = ctx.enter_context(tc.tile_pool(name="norm", bufs=3))
        osb_pool = ctx.enter_context(tc.tile_pool(name="osb", bufs=2))
        psum_mm = ctx.enter_context(tc.tile_pool(name="psum_mm", bufs=4,
                                                 space="PSUM"))
        psum_attn = ctx.enter_context(
            tc.tile_pool(name="psum_attn", bufs=2, space="PSUM")
        )
        psum_tp = ctx.enter_context(
            tc.tile_pool(name="psum_tp", bufs=2, space="PSUM")
        )

        ident_f = const.tile([128, 128], F32, tag="identf")
        make_identity(nc, ident_f[:])
        ident = const.tile([128, 128], MMDT, tag="ident")
        nc.vector.tensor_copy(ident[:], ident_f[:])

        krot_sb = const.tile([128, T], MMDT, tag="krot")
        # V~ tiles: [s, 0:128] = V^T tile, [s, 128] = ones (denominator col)
        v_sb = const.tile([128, ND, VW], MMDT, tag="v")
        nc.vector.memset(v_sb[:, :, 128:129], 1.0)
        wq_all = const.tile([128, ND, GH * H], MMDT, tag="wq")
        wk_all = const.tile([128, ND, H], MMDT, tag="wk")
        wv_all = const.tile([128, ND, H], MMDT, tag="wv")
        wo_all = const.tile([128, GH, D], MMDT, tag="wo")

        def rope(dst, src_psum, cc_t, ss_t):
            # dst = src * cc + swap_halves(src) * ss
            tmp1 = work.tile([128, 512], F32, tag="scratch")
            tmp2 = work.tile([128, 512], F32, tag="scratch")
            nc.vector.tensor_mul(tmp1[0:64, :], src_psum[64:128, :], ss_t[0:64, :])
            nc.vector.tensor_mul(tmp1[64:128, :], src_psum[0:64, :], ss_t[64:128, :])
            nc.vector.tensor_mul(tmp2[:], src_psum[:], cc_t[:])
            nc.vector.tensor_add(dst, tmp1[:], tmp2[:])

        for J in range(NJ):
            tsl = slice(J * 512, (J + 1) * 512)

            xq_t = xq_pool.tile([128, ND, 512], MMDT, tag="xq", name=f"xq{J}")
            if J == 0:
                # split the J0 prologue so the first Q matmuls' inputs land
                # first; interleave wq with xq per quarter
                for q in range(4):
                    qs = slice(q * 4, (q + 1) * 4)
                    nc.sync.dma_start(xq_t[:, qs, :], xq_src[:, qs, tsl])
                    nc.sync.dma_start(wq_all[:, qs, :], wq_src[:, qs, :])
            else:
                for q in range(ND // XG):
                    qs = slice(q * XG, (q + 1) * XG)
                    nc.sync.dma_start(xq_t[:, qs, :], xq_src[:, qs, tsl])
            xkv_halves = []
            if J == 0:
                nc.sync.dma_start(wk_all[:], wk_src[:])
                nc.sync.dma_start(wv_all[:], wv_src[:])
            for q in range(2):
                xh = xkv_pool.tile([128, 8, 512], MMDT, tag="xkv",
                                   name=f"xkv{J}_{q}")
                nc.sync.dma_start(xh[:], xkv_src[:, q * 8:(q + 1) * 8, tsl])
                xkv_halves.append(xh)
            tab_t = tab_pool.tile([128, 4, 512], F32, tag="tab", name=f"tab{J}")
            nc.sync.dma_start(tab_t[:], tab_src[:, :, tsl])
            ccq_t, ssq_t = tab_t[:, 0, :], tab_t[:, 1, :]
            cck_t, ssk_t = tab_t[:, 2, :], tab_t[:, 3, :]

            # ---- Q projection: sequential heads ----
            qrot = []
            for h in range(GH):
                qps = psum_mm.tile([128, 512], F32, tag="mm", name=f"qps{J}_{h}")
                for kd in range(ND):
                    nc.tensor.matmul(
                        qps[:], wq_all[:, kd, h * 128:(h + 1) * 128],
                        xq_t[:, kd, :], start=(kd == 0), stop=(kd == ND - 1),
                        skip_group_check=True)
                qr = qrot_pool.tile([128, 512], MMDT, tag="qrot",
                                    name=f"qrot{J}_{h}")
                rope(qr[:], qps[:], ccq_t, ssq_t)
                qrot.append(qr)

            # ---- K and V projections ----
            kps = psum_mm.tile([128, 512], F32, tag="mm")
            vps = psum_mm.tile([128, 512], F32, tag="mm")
            for q in range(2):
                xkv_t = xkv_halves[q]
                for kq in range(8):
                    kd = q * 8 + kq
                    nc.tensor.matmul(kps[:], wk_all[:, kd, :], xkv_t[:, kq, :],
                                     start=(kd == 0), stop=(kd == ND - 1),
                                     skip_group_check=True)
                    nc.tensor.matmul(vps[:], wv_all[:, kd, :], xkv_t[:, kq, :],
                                     start=(kd == 0), stop=(kd == ND - 1),
                                     skip_group_check=True)
            rope(krot_sb[:, tsl], kps[:], cck_t, ssk_t)

            # V: [h, t] -> PE-transpose -> v_sb tiles [s, h] (+ ones col)
            vt_sb = work.tile([128, 512], MMDT, tag="scratch")
            nc.vector.tensor_copy(vt_sb[:], vps[:])
            for st in range(4):
                tp = psum_tp.tile([128, 128], MMDT, tag="tp")
                nc.tensor.transpose(tp[:], vt_sb[:, st * 128:(st + 1) * 128],
                                    ident[:])
                s_tile = J * 4 + st
                nc.vector.tensor_copy(v_sb[:, s_tile, 0:128], tp[:])

            if J == 0:
                nc.sync.dma_start(wo_all[:], wo_src[:])

            # ---- SDPA for chunk J, all 4 heads ----
            attnT = []
            nk = 4 * J + 4
            for h in range(GH):
                pts = []
                for k in range(nk):
                    o = k - 4 * J  # >= 0 on diagonal tiles
                    off = max(0, o) * 128
                    sc = psum_mm.tile([128, 512], F32, tag="mm",
                                      name=f"sc{J}_{h}_{k}")
                    nc.tensor.matmul(sc[:, off:], krot_sb[:, k * 128:(k + 1) * 128],
                                     qrot[h][:, off:], start=True, stop=True)
                    pt = probs_pool.tile([128, 512], MMDT, tag="probs",
                                         name=f"pt{J}_{h}_{k}")
                    nc.scalar.activation(pt[:, off:], sc[:, off:],
                                         mybir.ActivationFunctionType.Exp,
                                         scale=SCALE)
                    if o >= 0:
                        # zero the causally-masked triangle (and stale cols)
                        nc.gpsimd.affine_select(
                            out=pt[:], in_=pt[:],
                            compare_op=mybir.AluOpType.is_ge,
                            fill=0.0, base=-o * 128,
                            pattern=[[1, 512]], channel_multiplier=-1)
                    pts.append(pt)

                at_h = attnt_pool.tile([128, 512], MMDT, tag="attnt",
                                       name=f"attnT{J}_{h}")
                for tt in range(4):
                    last_k = min(nk - 1, 4 * J + tt)
                    aps = psum_attn.tile([128, 132], F32, tag="attn",
                                         name=f"aps{J}_{h}_{tt}")
                    for k in range(last_k + 1):
                        nc.tensor.matmul(
                            aps[:, 0:129],
                            pts[k][:, tt * 128:(tt + 1) * 128],
                            v_sb[:, k, 0:129],
                            start=(k == 0), stop=(k == last_k))
                    rl = norm_pool.tile([128, 1], F32, tag="rl")
                    nc.vector.reciprocal(rl[:], aps[:, 128:129])
                    anorm = norm_pool.tile([128, 128], MMDT, tag="anorm")
                    nc.vector.tensor_scalar_mul(anorm[:], aps[:, 0:128], rl[:])
                    tp2 = psum_tp.tile([128, 128], MMDT, tag="tp")
                    nc.tensor.transpose(tp2[:], anorm[:], ident[:])
                    nc.vector.tensor_copy(at_h[:, tt * 128:(tt + 1) * 128],
                                          tp2[:])
                attnT.append(at_h)

            # ---- O projection for chunk J ----
            for tt in range(4):
                csl = slice(tt * 128, (tt + 1) * 128)
                for djp in range(2):  # 2 d-chunks of 512 -> one DMA each pair
                    ot = osb_pool.tile([128, 1024], F32, tag="osb",
                                       name=f"ot{J}_{tt}_{djp}")
                    for dje in range(2):
                        dj = djp * 2 + dje
                        ops = psum_tp.tile([128, 512], F32, tag="tp")
                        for h in range(GH):
                            nc.tensor.matmul(
                                ops[:], attnT[h][:, csl],
                                wo_all[:, h, dj * 512:(dj + 1) * 512],
                                start=(h == 0), stop=(h == GH - 1))
                        nc.scalar.copy(
                            ot[:, dje * 512:(dje + 1) * 512], ops[:])
                    # ACT-triggered HWDGE: output stores stay out of the
                    # sync-engine input-prefetch stream
                    nc.scalar.dma_start(
                        out[J * 512 + tt * 128:J * 512 + (tt + 1) * 128,
                            djp * 1024:(djp + 1) * 1024], ot[:])


def _rope_tables(positions):
    # positions: [T] int -> cc [128, T] = [cos; cos], ss [128, T] = [-sin; sin]
    half = H // 2
    fraction = 2.0 * np.arange(half, dtype=np.float64) / H
    timescale = MIN_TS * (MAX_TS / MIN_TS) ** fraction
    sinusoid = positions.astype(np.float64)[None, :] / timescale[:, None]
    sin = np.sin(sinusoid)
    cos = np.cos(sinusoid)
    cc = np.concatenate([cos, cos], axis=0).astype(np.float32)
    ss = np.concatenate([-sin, sin], axis=0).astype(np.float32)
    return cc, ss


def kernel(Xq, Xkv, q_positions, kv_positions, Wq, Wk, Wv, Wo):
    global _CACHED_NC, _last_in_maps
    if _CACHED_NC is None:
        _CACHED_NC = _build_core_program()
    nc = _CACHED_NC

    bf16 = mybir.dt.np(BF16)
    Xq = np.asarray(Xq, dtype=np.float32)
    Xkv = np.asarray(Xkv, dtype=np.float32)
    Wq = np.asarray(Wq, dtype=np.float32)
    Wk = np.asarray(Wk, dtype=np.float32)
    Wv = np.asarray(Wv, dtype=np.float32)
    Wo = np.asarray(Wo, dtype=np.float32)
    q_positions = np.asarray(q_positions)
    kv_positions = np.asarray(kv_positions)

    in_maps = []
    for c in range(8):
        b, g = c // 4, c % 4
        ccq, ssq = _rope_tables(q_positions[b])
        cck, ssk = _rope_tables(kv_positions[b])
        tabs = np.ascontiguousarray(
            np.concatenate([ccq, ssq, cck, ssk], axis=1))
        in_maps.append({
            "xqT": np.ascontiguousarray(Xq[b].T).astype(bf16),
            "xkvT": np.ascontiguousarray(Xkv[b].T).astype(bf16),
            "wq": np.ascontiguousarray(
                Wq[:, g * GH:(g + 1) * GH, :].reshape(D, GH * H)).astype(bf16),
            "wk": np.ascontiguousarray(Wk[:, g, :]).astype(bf16),
            "wv": np.ascontiguousarray(Wv[:, g, :]).astype(bf16),
            "wo": np.ascontiguousarray(
                Wo[g * GH:(g + 1) * GH].reshape(GH * H, D)).astype(bf16),
            "tabs": tabs,
        })

    _last_in_maps = in_maps

    res = run_bass_kernel_spmd(nc, in_maps, list(range(8)))

    outp = np.zeros((B, T, D), dtype=np.float64)
    for c in range(8):
        outp[c // 4] += res.results[c]["out"].astype(np.float64)
    return outp.astype(np.float32)


# revision 4
# speedup vs baseline: 8.9705x; 2.5659x over previous
"""GQA causal attention (B=2, T=2048, D=2048, N=16 q-heads, K=4 kv-heads, H=128)
on 8 Trainium2 NeuronCores.

Sharding: core c -> (batch b = c // 4, kv-head g = c % 4). Each core owns one
batch element and one GQA group (1 kv head + its 4 query heads) and computes
the full pipeline for that shard: Q/K/V projections, RoPE, causal SDPA, and
the O-projection partial over its 4 heads. The host pre-transposes activations
to [D, T] in bf16, precomputes RoPE sin/cos tables, and sums the 4 per-core
O-projection partials of each batch afterwards.

Device design notes (all matmuls bf16, 1 cycle/row on the PE at any free dim):
  - qT/kT live [head_dim, T]; scores are computed transposed [s, t] so the
    softmax denominator and PV contraction both run over the partition dim.
  - Causal masking: diagonal score tiles compute only the live columns
    (free-dim offset) and the probs triangle is zeroed by a Pool-engine
    affine_select after the exp - no mask adds on the Vector engine.
  - PV is computed with probs as the stationary operand and V~ = [V^T | 1]
    (129 columns) as the moving operand, so each [t,129] PSUM tile carries
    the attention numerator AND the softmax denominator in column 128:
    no separate row-sum matmuls. Normalization is a per-partition
    reciprocal+scale on the [t,h] tile, then a PE transpose back to [h,t]
    for the O projection.
  - DMAs are consolidated with 3D access patterns; output stores go through
    ACT-triggered HWDGE to stay out of the sync-engine input stream.
"""

import sys

for _p in ("/opt/trn_rl_repo", "/root/.axon_site/_ro/trn_rl_repo"):
    if _p not in sys.path:
        sys.path.append(_p)

import numpy as np

import concourse.bass as bass
import concourse.mybir as mybir
import concourse.tile as tile
from concourse import bacc
from concourse.bass_utils import run_bass_kernel_spmd
from concourse.masks import make_identity

B, T, D = 2, 2048, 2048
N_HEADS, K_HEADS, H = 16, 4, 128
GH = N_HEADS // K_HEADS          # 4 query heads per core
MIN_TS, MAX_TS = 1.0, 10000.0
NJ = T // 512                    # 4 column chunks of 512
ND = D // 128                    # 16 contraction chunks
SCALE = 1.0 / float(np.sqrt(H))

F32 = mybir.dt.float32
BF16 = mybir.dt.bfloat16
MMDT = BF16

XG = 4    # d-chunks per X dma_start
VW = 132  # v-tile stride: 128 V^T cols + 1 ones col + 3 pad

_CACHED_NC = None
_last_in_maps = None


def _build_core_program():
    nc = bacc.Bacc("TRN2", target_bir_lowering=False, debug=False, num_devices=8)

    xqT = nc.dram_tensor("xqT", [D, T], MMDT, kind="ExternalInput").ap()
    xkvT = nc.dram_tensor("xkvT", [D, T], MMDT, kind="ExternalInput").ap()
    wq = nc.dram_tensor("wq", [D, GH * H], MMDT, kind="ExternalInput").ap()
    wk = nc.dram_tensor("wk", [D, H], MMDT, kind="ExternalInput").ap()
    wv = nc.dram_tensor("wv", [D, H], MMDT, kind="ExternalInput").ap()
    wo = nc.dram_tensor("wo", [GH * H, D], MMDT, kind="ExternalInput").ap()
    tabs = nc.dram_tensor("tabs", [128, 4 * T], F32, kind="ExternalInput").ap()
    out = nc.dram_tensor("out", [T, D], F32, kind="ExternalOutput").ap()

    with tile.TileContext(nc) as tc:
        _emit(tc, nc, xqT, xkvT, wq, wk, wv, wo, tabs, out)
    nc.compile()
    return nc


def _emit(tc, nc, xqT, xkvT, wq, wk, wv, wo, tabs, out):
    from contextlib import ExitStack

    # 3D source views: [partition 128, d-chunk, col]
    xq_src = xqT.rearrange("(kd p) t -> p kd t", p=128)
    xkv_src = xkvT.rearrange("(kd p) t -> p kd t", p=128)
    wq_src = wq.rearrange("(kd p) n -> p kd n", p=128)
    wk_src = wk.rearrange("(kd p) n -> p kd n", p=128)
    wv_src = wv.rearrange("(kd p) n -> p kd n", p=128)
    wo_src = wo.rearrange("(h p) d -> p h d", p=128)
    tab_src = tabs.rearrange("p (i t) -> p i t", i=4)

    with ExitStack() as ctx:
        const = ctx.enter_context(tc.tile_pool(name="const", bufs=1))
        xq_pool = ctx.enter_context(tc.tile_pool(name="xq", bufs=2))
        xkv_pool = ctx.enter_context(tc.tile_pool(name="xkv", bufs=4))
        tab_pool = ctx.enter_context(tc.tile_pool(name="tab", bufs=2))
        qrot_pool = ctx.enter_context(tc.tile_pool(name="qrot", bufs=4))
        attnt_pool = ctx.enter_context(tc.tile_pool(name="attnt", bufs=5))
        probs_pool = ctx.enter_context(tc.tile_pool(name="probs", bufs=18))
        work = ctx.enter_context(tc.tile_pool(name="work", bufs=4))
        norm_pool = ctx.enter_context(tc.tile_pool(name="norm", bufs=3))
        osb_pool = ctx.enter_context(tc.tile_pool(name="osb", bufs=2))
        psum_mm = ctx.enter_context(tc.tile_pool(name="psum_mm", bufs=4,
                                                 space="PSUM"))
        psum_attn = ctx.enter_context(
            tc.tile_pool(name="psum_attn", bufs=2, space="PSUM")
        )
        psum_tp = ctx.enter_context(
            tc.tile_pool(name="psum_tp", bufs=2, space="PSUM")
        )

        ident_f = const.tile([128, 128], F32, tag="identf")
        make_identity(nc, ident_f[:])
        ident = const.tile([128, 128], MMDT, tag="ident")
        nc.vector.tensor_copy(ident[:], ident_f[:])

        krot_sb = const.tile([128, T], MMDT, tag="krot")
        # V~ tiles: [s, 0:128] = V^T tile, [s, 128] = ones (denominator col)
        v_sb = const.tile([128, ND, VW], MMDT, tag="v")
        nc.vector.memset(v_sb[:, :, 128:129], 1.0)
        wq_all = const.tile([128, ND, GH * H], MMDT, tag="wq")
        wk_all = const.tile([128, ND, H], MMDT, tag="wk")
        wv_all = const.tile([128, ND, H], MMDT, tag="wv")
        wo_all = const.tile([128, GH, D], MMDT, tag="wo")

        def rope(dst, src_psum, cc_t, ss_t):
            # dst = src * cc + swap_halves(src) * ss
            tmp1 = work.tile([128, 512], F32, tag="scratch")
            tmp2 = work.tile([128, 512], F32, tag="scratch")
            nc.vector.tensor_mul(tmp1[0:64, :], src_psum[64:128, :], ss_t[0:64, :])
            nc.vector.tensor_mul(tmp1[64:128, :], src_psum[0:64, :], ss_t[64:128, :])
            nc.vector.tensor_mul(tmp2[:], src_psum[:], cc_t[:])
            nc.vector.tensor_add(dst, tmp1[:], tmp2[:])

        for J in range(NJ):
            tsl = slice(J * 512, (J + 1) * 512)

            xq_t = xq_pool.tile([128, ND, 512], MMDT, tag="xq", name=f"xq{J}")
            if J == 0:
                # split the J0 prologue so the first Q matmuls' inputs land
                # first; interleave wq with xq per quarter
                for q in range(4):
                    qs = slice(q * 4, (q + 1) * 4)
                    nc.sync.dma_start(xq_t[:, qs, :], xq_src[:, qs, tsl])
                    nc.sync.dma_start(wq_all[:, qs, :], wq_src[:, qs, :])
            else:
                for q in range(ND // XG):
                    qs = slice(q * XG, (q + 1) * XG)
                    nc.sync.dma_start(xq_t[:, qs, :], xq_src[:, qs, tsl])
            xkv_halves = []
            if J == 0:
                nc.sync.dma_start(wk_all[:], wk_src[:])
                nc.sync.dma_start(wv_all[:], wv_src[:])
            for q in range(2):
                xh = xkv_pool.tile([128, 8, 512], MMDT, tag="xkv",
                                   name=f"xkv{J}_{q}")
                nc.sync.dma_start(xh[:], xkv_src[:, q * 8:(q + 1) * 8, tsl])
                xkv_halves.append(xh)
            tab_t = tab_pool.tile([128, 4, 512], F32, tag="tab", name=f"tab{J}")
            nc.sync.dma_start(tab_t[:], tab_src[:, :, tsl])
            ccq_t, ssq_t = tab_t[:, 0, :], tab_t[:, 1, :]
            cck_t, ssk_t = tab_t[:, 2, :], tab_t[:, 3, :]

            # ---- Q projection: sequential heads ----
            qrot = []
            for h in range(GH):
                qps = psum_mm.tile([128, 512], F32, tag="mm", name=f"qps{J}_{h}")
                for kd in range(ND):
                    nc.tensor.matmul(
                        qps[:], wq_all[:, kd, h * 128:(h + 1) * 128],
                        xq_t[:, kd, :], start=(kd == 0), stop=(kd == ND - 1),
                        skip_group_check=True)
                qr = qrot_pool.tile([128, 512], MMDT, tag="qrot",
                                    name=f"qrot{J}_{h}")
                rope(qr[:], qps[:], ccq_t, ssq_t)
                qrot.append(qr)

            # ---- K and V projections ----
            kps = psum_mm.tile([128, 512], F32, tag="mm")
            vps = psum_mm.tile([128, 512], F32, tag="mm")
            for q in range(2):
                xkv_t = xkv_halves[q]
                for kq in range(8):
                    kd = q * 8 + kq
                    nc.tensor.matmul(kps[:], wk_all[:, kd, :], xkv_t[:, kq, :],
                                     start=(kd == 0), stop=(kd == ND - 1),
                                     skip_group_check=True)
                    nc.tensor.matmul(vps[:], wv_all[:, kd, :], xkv_t[:, kq, :],
                                     start=(kd == 0), stop=(kd == ND - 1),
                                     skip_group_check=True)
            rope(krot_sb[:, tsl], kps[:], cck_t, ssk_t)

            # V: [h, t] -> PE-transpose -> v_sb tiles [s, h] (+ ones col)
            vt_sb = work.tile([128, 512], MMDT, tag="scratch")
            nc.vector.tensor_copy(vt_sb[:], vps[:])
            for st in range(4):
                tp = psum_tp.tile([128, 128], MMDT, tag="tp")
                nc.tensor.transpose(tp[:], vt_sb[:, st * 128:(st + 1) * 128],
                                    ident[:])
                s_tile = J * 4 + st
                nc.vector.tensor_copy(v_sb[:, s_tile, 0:128], tp[:])

            if J == 0:
                nc.sync.dma_start(wo_all[:], wo_src[:])

            # ---- SDPA for chunk J, all 4 heads ----
            attnT = []
            nk = 4 * J + 4
            for h in range(GH):
                pts = []
                for k in range(nk):
                    o = k - 4 * J  # >= 0 on diagonal tiles
                    off = max(0, o) * 128
                    sc = psum_mm.tile([128, 512], F32, tag="mm",
                                      name=f"sc{J}_{h}_{k}")
                    nc.tensor.matmul(sc[:, off:], krot_sb[:, k * 128:(k + 1) * 128],
                                     qrot[h][:, off:], start=True, stop=True)
                    pt = probs_pool.tile([128, 512], MMDT, tag="probs",
                                         name=f"pt{J}_{h}_{k}")
                    nc.scalar.activation(pt[:, off:], sc[:, off:],
                                         mybir.ActivationFunctionType.Exp,
                                         scale=SCALE)
                    if o >= 0:
                        # zero the causally-masked triangle (and stale cols)
                        nc.gpsimd.affine_select(
                            out=pt[:], in_=pt[:],
                            compare_op=mybir.AluOpType.is_ge,
                            fill=0.0, base=-o * 128,
                            pattern=[[1, 512]], channel_multiplier=-1)
                    pts.append(pt)

                at_h = attnt_pool.tile([128, 512], MMDT, tag="attnt",
                                       name=f"attnT{J}_{h}")
                for tt in range(4):
                    last_k = min(nk - 1, 4 * J + tt)
                    aps = psum_attn.tile([128, 132], F32, tag="attn",
                                         name=f"aps{J}_{h}_{tt}")
                    for k in range(last_k + 1):
                        nc.tensor.matmul(
                            aps[:, 0:129],
                            pts[k][:, tt * 128:(tt + 1) * 128],
                            v_sb[:, k, 0:129],
                            start=(k == 0), stop=(k == last_k))
                    rl = norm_pool.tile([128, 1], F32, tag="rl")
                    nc.vector.reciprocal(rl[:], aps[:, 128:129])
                    anorm = norm_pool.tile([128, 128], MMDT, tag="anorm")
                    nc.vector.tensor_scalar_mul(anorm[:], aps[:, 0:128], rl[:])
                    tp2 = psum_tp.tile([128, 128], MMDT, tag="tp")
                    nc.tensor.transpose(tp2[:], anorm[:], ident[:])
                    nc.vector.tensor_copy(at_h[:, tt * 128:(tt + 1) * 128],
                                          tp2[:])
                attnT.append(at_h)

            # ---- O projection for chunk J ----
            for tt in range(4):
                csl = slice(tt * 128, (tt + 1) * 128)
                for djp in range(2):  # 2 d-chunks of 512 -> one DMA each pair
                    ot = osb_pool.tile([128, 1024], F32, tag="osb",
                                       name=f"ot{J}_{tt}_{djp}")
                    for dje in range(2):
                        dj = djp * 2 + dje
                        ops = psum_tp.tile([128, 512], F32, tag="tp")
                        for h in range(GH):
                            nc.tensor.matmul(
                                ops[:], attnT[h][:, csl],
                                wo_all[:, h, dj * 512:(dj + 1) * 512],
                                start=(h == 0), stop=(h == GH - 1))
                        nc.scalar.copy(
                            ot[:, dje * 512:(dje + 1) * 512], ops[:])
                    # ACT-triggered HWDGE: output stores stay out of the
                    # sync-engine input-prefetch stream
                    nc.scalar.dma_start(
                        out[J * 512 + tt * 128:J * 512 + (tt + 1) * 128,
                            djp * 1024:(djp + 1) * 1024], ot[:])


def _rope_tables(positions):
    # positions: [T] int -> cc [128, T] = [cos; cos], ss [128, T] = [-sin; sin]
    half = H // 2
    fraction = 2.0 * np.arange(half, dtype=np.float64) / H
    timescale = MIN_TS * (MAX_TS / MIN_TS) ** fraction
    sinusoid = positions.astype(np.float64)[None, :] / timescale[:, None]
    sin = np.sin(sinusoid)
    cos = np.cos(sinusoid)
    cc = np.concatenate([cos, cos], axis=0).astype(np.float32)
    ss = np.concatenate([-sin, sin], axis=0).astype(np.float32)
    return cc, ss


def kernel(Xq, Xkv, q_positions, kv_positions, Wq, Wk, Wv, Wo):
    global _CACHED_NC, _last_in_maps
    if _CACHED_NC is None:
        _CACHED_NC = _build_core_program()
    nc = _CACHED_NC

    bf16 = mybir.dt.np(BF16)
    Xq = np.asarray(Xq, dtype=np.float32)
    Xkv = np.asarray(Xkv, dtype=np.float32)
    Wq = np.asarray(Wq, dtype=np.float32)
    Wk = np.asarray(Wk, dtype=np.float32)
    Wv = np.asarray(Wv, dtype=np.float32)
    Wo = np.asarray(Wo, dtype=np.float32)
    q_positions = np.asarray(q_positions)
    kv_positions = np.asarray(kv_positions)

    in_maps = []
    for c in range(8):
        b, g = c // 4, c % 4
        ccq, ssq = _rope_tables(q_positions[b])
        cck, ssk = _rope_tables(kv_positions[b])
        tabs = np.ascontiguousarray(
            np.concatenate([ccq, ssq, cck, ssk], axis=1))
        in_maps.append({
            "xqT": np.ascontiguousarray(Xq[b].T).astype(bf16),
            "xkvT": np.ascontiguousarray(Xkv[b].T).astype(bf16),
            "wq": np.ascontiguousarray(
                Wq[:, g * GH:(g + 1) * GH, :].reshape(D, GH * H)).astype(bf16),
            "wk": np.ascontiguousarray(Wk[:, g, :]).astype(bf16),
            "wv": np.ascontiguousarray(Wv[:, g, :]).astype(bf16),
            "wo": np.ascontiguousarray(
                Wo[g * GH:(g + 1) * GH].reshape(GH * H, D)).astype(bf16),
            "tabs": tabs,
        })

    _last_in_maps = in_maps

    res = run_bass_kernel_spmd(nc, in_maps, list(range(8)))

    outp = np.zeros((B, T, D), dtype=np.float64)
    for c in range(8):
        outp[c // 4] += res.results[c]["out"].astype(np.float64)
    return outp.astype(np.float32)
